# revision 3
# baseline (speedup 1.0000x reference)
"""Causal self-attention (RoPE on k/v) TRN2 Bass kernel.

Sharding: core i handles batch b = i//2 and 8 heads hs = 8*(i%2).
Each core computes qkv projection for its (batch, head-group), RoPE on
k and v, causal attention, and a partial output projection y^T with its
W_proj row-block.  Host sums the two partials per batch and adds b_proj.

Layouts on device (per core):
  xT   [C, T]     x[b]^T (host-transposed)
  qkT  [2048, T]  rows 0-1023 = q^T (head-major, perm'd d), 1024-2047 = rope(k)^T
  vr   [T, 1024]  rope(v), natural layout
  oT   [1024, T]  attention out^T
  yT   [C, T]     partial out-proj (output)

Head-dim permutation (even dims first) turns RoPE's interleaved
even/odd pairs into contiguous 64-row/col halves; W_attn columns and
W_proj rows are permuted correspondingly on host, which leaves the
attention math invariant.

All matmul operands are float32r (fp32 bits, 1 cycle/row on PE at
N>=256 vs 4 for plain fp32; measured relerr 1.5e-4 vs fp64, identical
to the fp32 matmul path on this HW).
"""
import sys

sys.path.insert(0, "/opt/trn_rl_repo")

import numpy as np

import concourse.bass as bass  # noqa: F401
import concourse.mybir as mybir
import concourse.tile as tile
from concourse import bacc
from concourse.bass_utils import run_bass_kernel_spmd

B, T, C, H = 4, 2048, 2048, 16
HD = 128
HC = 8  # heads per core
NCORES = 8
F32 = mybir.dt.float32
F32R = mybir.dt.float32r
SCALE = float(1.0 / np.sqrt(HD))

_CACHE = {}


def _build_nc():
    nc = bacc.Bacc(num_devices=NCORES)

    xT = nc.dram_tensor("xT", [C, T], F32R, kind="ExternalInput")
    wqk = nc.dram_tensor("wqk", [C, 2048], F32R, kind="ExternalInput")
    bqk = nc.dram_tensor("bqk", [128, 16], F32, kind="ExternalInput")
    wv = nc.dram_tensor("wv", [C, 1024], F32R, kind="ExternalInput")
    bv = nc.dram_tensor("bv", [128, 1024], F32, kind="ExternalInput")
    wp = nc.dram_tensor("wp", [1024, C], F32R, kind="ExternalInput")
    rtab_u = nc.dram_tensor("rtab_u", [128, T], F32, kind="ExternalInput")
    rtab_v = nc.dram_tensor("rtab_v", [128, T], F32, kind="ExternalInput")
    cos4 = nc.dram_tensor("cos4", [T, 256], F32, kind="ExternalInput")
    sin4 = nc.dram_tensor("sin4", [T, 256], F32, kind="ExternalInput")
    masks = nc.dram_tensor("masks", [128, 2048], F32, kind="ExternalInput")
    yT = nc.dram_tensor("yT", [C, T], F32, kind="ExternalOutput")

    qkT = nc.dram_tensor("qkT", [2048, T], F32R)
    vr_d = nc.dram_tensor("vr_d", [T, 1024], F32R)
    oT_d = nc.dram_tensor("oT_d", [1024, T], F32R)

    with tile.TileContext(nc) as tc:
        if True:
            # ---------------- Phase A1: q^T and rope(k)^T ----------------
            # xT resident (128KB/partition); W_qk streamed per column tile.
            with tc.tile_pool(name="xt", bufs=1) as xtp, \
                 tc.tile_pool(name="a1tab", bufs=1) as atabp, \
                 tc.tile_pool(name="wblk", bufs=2) as wbp, \
                 tc.tile_pool(name="qko", bufs=4) as qkop, \
                 tc.tile_pool(name="ktmp", bufs=2) as ktp, \
                 tc.tile_pool(name="psA1", bufs=6, space="PSUM") as psp:
                xt = xtp.tile([128, 16, T], F32R)
                nc.sync.dma_start(xt[:], xT.rearrange("(a p) t -> p a t", p=128))
                ut = atabp.tile([128, T], F32)
                nc.sync.dma_start(ut[:], rtab_u[:, :])
                vt_tab = atabp.tile([128, T], F32)
                nc.sync.dma_start(vt_tab[:], rtab_v[:, :])
                bqk_t = atabp.tile([128, 16], F32)
                nc.sync.dma_start(bqk_t[:], bqk[:, :])

                wqk_r = wqk.rearrange("(a p) j -> p a j", p=128)
                for jt in range(16):
                    wblk = wbp.tile([128, 16, 128], F32R, tag="wblk")
                    nc.sync.dma_start(
                        wblk[:], wqk_r[:, :, jt * 128:(jt + 1) * 128])
                    for tb in range(4):
                        ts = bass.ts(tb, 512)
                        ps = psp.tile([128, 512], F32, tag="ps")
                        for c in range(16):
                            nc.tensor.matmul(
                                ps[:], wblk[:, c], xt[:, c, ts],
                                start=(c == 0), stop=(c == 15))
                        if jt < 8:
                            qo = qkop.tile([128, 512], F32R, tag="qko")
                            nc.vector.tensor_scalar_add(
                                qo[:], ps[:], bqk_t[:, jt:jt + 1])
                            nc.sync.dma_start(
                                qkT[jt * 128:(jt + 1) * 128, ts], qo[:])
                        else:
                            kt = ktp.tile([128, 512], F32, tag="kt")
                            nc.vector.tensor_scalar_add(
                                kt[:], ps[:], bqk_t[:, jt:jt + 1])
                            kts = ktp.tile([128, 512], F32, tag="kts")
                            nc.sync.dma_start(kts[0:64, :], kt[64:128, :])
                            nc.sync.dma_start(kts[64:128, :], kt[0:64, :])
                            m1 = ktp.tile([128, 512], F32, tag="m1")
                            nc.vector.tensor_mul(m1[:], kt[:], ut[:, ts])
                            m2 = ktp.tile([128, 512], F32, tag="m2")
                            nc.vector.tensor_mul(
                                m2[:], kts[:], vt_tab[:, ts])
                            ko = qkop.tile([128, 512], F32R, tag="qko")
                            nc.vector.tensor_add(ko[:], m1[:], m2[:])
                            nc.sync.dma_start(
                                qkT[jt * 128:(jt + 1) * 128, ts], ko[:])

            # ---------------- Phase A2: v natural + rope ----------------
            # Full Wv resident (64KB/partition); xT tiles streamed.
            with tc.tile_pool(name="wvf", bufs=1) as wvp, \
                 tc.tile_pool(name="xa", bufs=3) as xap, \
                 tc.tile_pool(name="vtab", bufs=1) as vtabp, \
                 tc.tile_pool(name="vro", bufs=4) as vrop, \
                 tc.tile_pool(name="vtmp", bufs=2) as vtp, \
                 tc.tile_pool(name="psA2", bufs=6, space="PSUM") as psp:
                wvf = wvp.tile([128, 16, 1024], F32R)
                nc.sync.dma_start(wvf[:], wv.rearrange("(a p) d -> p a d", p=128))
                bv_t = vtabp.tile([128, 1024], F32)
                nc.sync.dma_start(bv_t[:], bv[:, :])
                c4 = vtabp.tile([128, 16, 256], F32)
                nc.sync.dma_start(
                    c4[:], cos4.rearrange("(a p) i -> p a i", p=128))
                s4 = vtabp.tile([128, 16, 256], F32)
                nc.sync.dma_start(
                    s4[:], sin4.rearrange("(a p) i -> p a i", p=128))

                xT_r = xT.rearrange("(a p) t -> p a t", p=128)
                for tt in range(16):
                    xa = xap.tile([128, 16, 128], F32R, tag="xa")
                    nc.sync.dma_start(
                        xa[:], xT_r[:, :, bass.ts(tt, 128)])
                    for db in range(2):
                        ds = bass.ts(db, 512)
                        ps = psp.tile([128, 512], F32, tag="ps")
                        for c in range(16):
                            nc.tensor.matmul(
                                ps[:], xa[:, c],
                                wvf[:, c, ds], start=(c == 0), stop=(c == 15))
                        vtmp = vtp.tile([128, 512], F32, tag="vtmp")
                        nc.vector.tensor_add(vtmp[:], ps[:], bv_t[:, ds])
                        v3 = vtmp[:].rearrange("p (h d) -> p h d", h=4)
                        c43 = c4[:, tt].rearrange("p (h d) -> p h d", h=4)
                        s43 = s4[:, tt].rearrange("p (h d) -> p h d", h=4)
                        me = vtp.tile([128, 4, 64], F32, tag="me")
                        mo = vtp.tile([128, 4, 64], F32, tag="mo")
                        vro = vrop.tile([128, 512], F32R, tag="vro")
                        vr3 = vro[:].rearrange("p (h d) -> p h d", h=4)
                        nc.vector.tensor_mul(
                            me[:], v3[:, :, 0:64], c43[:, :, 0:64])
                        nc.vector.tensor_mul(
                            mo[:], v3[:, :, 64:128], s43[:, :, 0:64])
                        nc.vector.tensor_sub(
                            vr3[:, :, 0:64], me[:], mo[:])
                        nc.vector.tensor_mul(
                            me[:], v3[:, :, 0:64], s43[:, :, 0:64])
                        nc.vector.tensor_mul(
                            mo[:], v3[:, :, 64:128], c43[:, :, 0:64])
                        nc.vector.tensor_add(
                            vr3[:, :, 64:128], me[:], mo[:])
                        nc.sync.dma_start(
                            vr_d[bass.ts(tt, 128), ds], vro[:])

            # ---------------- Phase B: attention per head ----------------
            with tc.tile_pool(name="hk", bufs=2) as hkp, \
                 tc.tile_pool(name="hq", bufs=2) as hqp, \
                 tc.tile_pool(name="hv", bufs=2) as hvp, \
                 tc.tile_pool(name="pt", bufs=4) as ptp, \
                 tc.tile_pool(name="bsc", bufs=2) as bscp, \
                 tc.tile_pool(name="oo", bufs=2) as oop, \
                 tc.tile_pool(name="const", bufs=1) as constp, \
                 tc.tile_pool(name="psB", bufs=5, space="PSUM") as psp, \
                 tc.tile_pool(name="lps", bufs=1, space="PSUM") as lpsp, \
                 tc.tile_pool(name="ops", bufs=2, space="PSUM") as opsp:
                ones_f = constp.tile([128, 1], F32)
                nc.vector.memset(ones_f[:], 1.0)
                ones_t = constp.tile([128, 1], F32R)
                nc.vector.tensor_copy(ones_t[:], ones_f[:])
                masks_t = constp.tile([128, 4, 512], F32)
                nc.sync.dma_start(
                    masks_t[:], masks.rearrange("p (r i) -> p r i", r=4))
                vr_r = vr_d.rearrange("(jt p) d -> p jt d", p=128)
                for h in range(HC):
                    krh = hkp.tile([128, T], F32R, tag="krh")
                    nc.sync.dma_start(
                        krh[:], qkT[1024 + h * 128:1024 + (h + 1) * 128, :])
                    qh = hqp.tile([128, T], F32R, tag="qh")
                    nc.sync.dma_start(qh[:], qkT[h * 128:(h + 1) * 128, :])
                    vh = hvp.tile([128, 16, 128], F32R, tag="vh")
                    nc.sync.dma_start(
                        vh[:], vr_r[:, :, h * 128:(h + 1) * 128])

                    for ib in range(4):
                        isl = bass.ts(ib, 512)
                        nj = 4 * ib + 4
                        l_ps = lpsp.tile([1, 512], F32, tag="l")
                        o_ps = opsp.tile([128, 512], F32, tag="o")
                        pts = [None] * nj

                        def consume(jt):
                            pt = pts[jt]
                            nc.tensor.matmul(
                                l_ps[:], ones_t[:], pt[:],
                                start=(jt == 0), stop=(jt == nj - 1))
                            nc.tensor.matmul(
                                o_ps[:], vh[:, jt], pt[:],
                                start=(jt == 0), stop=(jt == nj - 1))

                        for jt in range(nj):
                            s_ps = psp.tile([128, 512], F32, tag="ps")
                            nc.tensor.matmul(
                                s_ps[:], krh[:, bass.ts(jt, 128)],
                                qh[:, isl], start=True, stop=True)
                            pt = ptp.tile([128, 512], F32R, tag="pt")
                            nc.scalar.activation(
                                pt[:], s_ps[:],
                                mybir.ActivationFunctionType.Exp, scale=SCALE)
                            if jt >= 4 * ib:
                                nc.vector.tensor_mul(
                                    pt[:], pt[:], masks_t[:, jt - 4 * ib])
                            pts[jt] = pt
                            if jt >= 1:
                                consume(jt - 1)
                        consume(nj - 1)

                        r_sb = bscp.tile([1, 512], F32, tag="r")
                        nc.vector.reciprocal(r_sb[:], l_ps[:])
                        rb = bscp.tile([128, 512], F32, tag="rb")
                        nc.gpsimd.partition_broadcast(rb[:], r_sb[:])
                        oo = oop.tile([128, 512], F32R, tag="oo")
                        nc.vector.tensor_mul(oo[:], o_ps[:], rb[:])
                        nc.sync.dma_start(
                            oT_d[h * 128:(h + 1) * 128, isl], oo[:])

            # ---------------- Phase C: out projection ----------------
            with tc.tile_pool(name="wpb", bufs=1) as wpp, \
                 tc.tile_pool(name="otb", bufs=2) as otbp, \
                 tc.tile_pool(name="yo", bufs=4) as yop, \
                 tc.tile_pool(name="psC", bufs=6, space="PSUM") as psp:
                wps = wpp.tile([128, 8, C], F32R)
                nc.sync.dma_start(
                    wps[:], wp.rearrange("(ht p) c -> p ht c", p=128))
                oT_r = oT_d.rearrange("(ht p) t -> p ht t", p=128)
                for tb in range(4):
                    ts = bass.ts(tb, 512)
                    otb = otbp.tile([128, 8, 512], F32R, tag="otb")
                    nc.sync.dma_start(otb[:], oT_r[:, :, ts])
                    for ct in range(16):
                        ps = psp.tile([128, 512], F32, tag="ps")
                        for ht in range(8):
                            nc.tensor.matmul(
                                ps[:], wps[:, ht, bass.ts(ct, 128)],
                                otb[:, ht], start=(ht == 0), stop=(ht == 7))
                        yo = yop.tile([128, 512], F32, tag="yo")
                        nc.vector.tensor_copy(yo[:], ps[:])
                        nc.sync.dma_start(
                            yT[ct * 128:(ct + 1) * 128, ts], yo[:])

    nc.compile()
    return nc


def _prep_inputs(x, freqs_cos, freqs_sin, W_attn, b_attn, W_proj):
    """Host-side sharding / layout prep.  Returns list of 8 in_maps."""
    perm = np.concatenate([np.arange(0, HD, 2), np.arange(1, HD, 2)])

    cosT = np.ascontiguousarray(freqs_cos.T)  # [64, T]
    sinT = np.ascontiguousarray(freqs_sin.T)
    rtab_u = np.concatenate([cosT, cosT], axis=0).astype(np.float32)
    rtab_v = np.concatenate([-sinT, sinT], axis=0).astype(np.float32)
    cos4 = np.tile(freqs_cos, (1, 4)).astype(np.float32)  # [T, 256]
    sin4 = np.tile(freqs_sin, (1, 4)).astype(np.float32)

    jj = np.arange(128)[:, None]
    ii = np.arange(512)[None, :]
    masks = np.concatenate(
        [((r * 128 + jj) <= ii).astype(np.float32) for r in range(4)],
        axis=1)  # [128, 2048]

    in_maps = []
    for core in range(NCORES):
        b = core // 2
        hs = HC * (core % 2)
        cols = np.concatenate(
            [g * HD + perm for g in range(hs, hs + HC)])  # [1024]

        wqk = np.concatenate(
            [W_attn[:, cols], W_attn[:, C + cols]], axis=1)
        bqk_flat = np.concatenate([b_attn[cols], b_attn[C + cols]])
        bqk = np.ascontiguousarray(
            bqk_flat.reshape(16, 128).T)  # [128, 16], bias[jt*128+p]
        wv = W_attn[:, 2 * C + cols]
        bv = np.broadcast_to(b_attn[2 * C + cols], (128, 1024))
        wp = W_proj[cols, :]

        in_maps.append({
            "xT": np.ascontiguousarray(x[b].T).astype(np.float32),
            "wqk": np.ascontiguousarray(wqk).astype(np.float32),
            "bqk": np.ascontiguousarray(bqk).astype(np.float32),
            "wv": np.ascontiguousarray(wv).astype(np.float32),
            "bv": np.ascontiguousarray(bv).astype(np.float32),
            "wp": np.ascontiguousarray(wp).astype(np.float32),
            "rtab_u": rtab_u,
            "rtab_v": rtab_v,
            "cos4": cos4,
            "sin4": sin4,
            "masks": np.ascontiguousarray(masks),
        })
    return in_maps


def kernel(x, freqs_cos, freqs_sin, mask, W_attn, b_attn, W_proj, b_proj,
           _return_results=False, _trace=False):
    x = np.asarray(x, dtype=np.float32)
    freqs_cos = np.asarray(freqs_cos, dtype=np.float32)
    freqs_sin = np.asarray(freqs_sin, dtype=np.float32)
    W_attn = np.asarray(W_attn, dtype=np.float32)
    b_attn = np.asarray(b_attn, dtype=np.float32)
    W_proj = np.asarray(W_proj, dtype=np.float32)
    b_proj = np.asarray(b_proj, dtype=np.float32)

    if "nc" not in _CACHE:
        _CACHE["nc"] = _build_nc()
    nc = _CACHE["nc"]

    in_maps = _prep_inputs(x, freqs_cos, freqs_sin, W_attn, b_attn, W_proj)
    res = run_bass_kernel_spmd(nc, in_maps, core_ids=list(range(NCORES)),
                               trace=_trace)

    out = np.empty((B, T, C), dtype=np.float32)
    for b in range(B):
        yt0 = res.results[2 * b]["yT"]
        yt1 = res.results[2 * b + 1]["yT"]
        out[b] = yt0.T + yt1.T + b_proj[None, :]
    if _return_results:
        return out, res
    return out



# revision 11
# speedup vs baseline: 1.7034x; 1.7034x over previous
"""Causal self-attention (RoPE on k/v) TRN2 Bass kernel.

Sharding: core i handles batch b = i//2 and 8 heads hs = 8*(i%2).
Each core computes qkv projection for its (batch, head-group), RoPE on
k and v, causal attention, and a partial output projection y^T with its
W_proj row-block.  Host sums the two partials per batch and adds b_proj.

Layouts on device (per core):
  xT   [C, T]     x[b]^T (host-transposed)
  qkT  [2048, T]  rows 0-1023 = q^T (head-major, perm'd d), 1024-2047 = rope(k)^T
  vr   [T, 1024]  rope(v), natural layout
  oT   [1024, T]  attention out^T
  yT   [C, T]     partial out-proj (output)

Head-dim permutation (even dims first) turns RoPE's interleaved
even/odd pairs into contiguous 64-row/col halves; W_attn columns and
W_proj rows are permuted correspondingly on host, which leaves the
attention math invariant.

All matmul operands are float32r (fp32 bits, 1 cycle/row on PE at
N>=256 vs 4 for plain fp32; measured relerr 1.5e-4 vs fp64, identical
to the fp32 matmul path on this HW).
"""
import sys

sys.path.insert(0, "/opt/trn_rl_repo")

import numpy as np

import concourse.bass as bass  # noqa: F401
import concourse.mybir as mybir
import concourse.tile as tile
from concourse import bacc
from concourse.bass_utils import run_bass_kernel_spmd

B, T, C, H = 4, 2048, 2048, 16
HD = 128
HC = 8  # heads per core
NCORES = 8
F32 = mybir.dt.float32
F32R = mybir.dt.float32r
SCALE = float(1.0 / np.sqrt(HD))

_CACHE = {}


def _build_nc():
    nc = bacc.Bacc(num_devices=NCORES)

    xT = nc.dram_tensor("xT", [C, T], F32R, kind="ExternalInput")
    wqk = nc.dram_tensor("wqk", [C, 2048], F32R, kind="ExternalInput")
    bqk = nc.dram_tensor("bqk", [128, 16], F32, kind="ExternalInput")
    wv = nc.dram_tensor("wv", [C, 1024], F32R, kind="ExternalInput")
    bv = nc.dram_tensor("bv", [128, 1024], F32, kind="ExternalInput")
    wp = nc.dram_tensor("wp", [1024, C], F32R, kind="ExternalInput")
    rtab_u = nc.dram_tensor("rtab_u", [128, T], F32, kind="ExternalInput")
    rtab_v = nc.dram_tensor("rtab_v", [128, T], F32, kind="ExternalInput")
    cos4 = nc.dram_tensor("cos4", [T, 256], F32, kind="ExternalInput")
    sin4 = nc.dram_tensor("sin4", [T, 256], F32, kind="ExternalInput")
    masks = nc.dram_tensor("masks", [128, 2048], F32, kind="ExternalInput")
    yT = nc.dram_tensor("yT", [C, T], F32, kind="ExternalOutput")

    qkT = nc.dram_tensor("qkT", [2048, T], F32R)
    vr_d = nc.dram_tensor("vr_d", [T, 1024], F32R)
    oT_d = nc.dram_tensor("oT_d", [1024, T], F32R)

    with tile.TileContext(nc) as tc:
        if True:
            # ---------------- Phase A1: q^T and rope(k)^T ----------------
            # xT resident (128KB/partition); W_qk streamed per column tile.
            with tc.tile_pool(name="xt", bufs=1) as xtp, \
                 tc.tile_pool(name="a1tab", bufs=1) as atabp, \
                 tc.tile_pool(name="wblk", bufs=2) as wbp, \
                 tc.tile_pool(name="qko", bufs=4) as qkop, \
                 tc.tile_pool(name="ktmp", bufs=2) as ktp, \
                 tc.tile_pool(name="psA1", bufs=6, space="PSUM") as psp:
                xt = xtp.tile([128, 16, T], F32R)
                xT_r16 = xT.rearrange("(a p) t -> p a t", p=128)
                for c in range(16):
                    nc.sync.dma_start(xt[:, c], xT_r16[:, c])
                ut = atabp.tile([128, T], F32)
                nc.sync.dma_start(ut[:], rtab_u[:, :])
                vt_tab = atabp.tile([128, T], F32)
                nc.sync.dma_start(vt_tab[:], rtab_v[:, :])
                bqk_t = atabp.tile([128, 16], F32)
                nc.sync.dma_start(bqk_t[:], bqk[:, :])

                wqk_r = wqk.rearrange("(a p) j -> p a j", p=128)
                for jt in range(16):
                    wblk = wbp.tile([128, 16, 128], F32R, tag="wblk")
                    nc.sync.dma_start(
                        wblk[:], wqk_r[:, :, jt * 128:(jt + 1) * 128])
                    for tb in range(4):
                        ts = bass.ts(tb, 512)
                        ps = psp.tile([128, 512], F32, tag="ps")
                        for c in range(16):
                            nc.tensor.matmul(
                                ps[:], wblk[:, c], xt[:, c, ts],
                                start=(c == 0), stop=(c == 15))
                        if jt < 8:
                            qo = qkop.tile([128, 512], F32R, tag="qko")
                            nc.vector.tensor_scalar_add(
                                qo[:], ps[:], bqk_t[:, jt:jt + 1])
                            nc.sync.dma_start(
                                qkT[jt * 128:(jt + 1) * 128, ts], qo[:])
                        else:
                            kt = ktp.tile([128, 512], F32, tag="kt")
                            nc.vector.tensor_scalar_add(
                                kt[:], ps[:], bqk_t[:, jt:jt + 1])
                            kts = ktp.tile([128, 512], F32, tag="kts")
                            nc.sync.dma_start(kts[0:64, :], kt[64:128, :])
                            nc.sync.dma_start(kts[64:128, :], kt[0:64, :])
                            m1 = ktp.tile([128, 512], F32, tag="m1")
                            nc.vector.tensor_mul(m1[:], kt[:], ut[:, ts])
                            m2 = ktp.tile([128, 512], F32, tag="m2")
                            nc.vector.tensor_mul(
                                m2[:], kts[:], vt_tab[:, ts])
                            ko = qkop.tile([128, 512], F32R, tag="qko")
                            nc.vector.tensor_add(ko[:], m1[:], m2[:])
                            nc.sync.dma_start(
                                qkT[jt * 128:(jt + 1) * 128, ts], ko[:])

            # ---------------- Phase A2: v natural + rope ----------------
            # Full Wv resident (64KB/partition); xT tiles streamed.
            with tc.tile_pool(name="wvf", bufs=1) as wvp, \
                 tc.tile_pool(name="xa", bufs=3) as xap, \
                 tc.tile_pool(name="vtab", bufs=1) as vtabp, \
                 tc.tile_pool(name="vro", bufs=4) as vrop, \
                 tc.tile_pool(name="vtmp", bufs=2) as vtp, \
                 tc.tile_pool(name="psA2", bufs=6, space="PSUM") as psp:
                wvf = wvp.tile([128, 16, 1024], F32R)
                wv_r16 = wv.rearrange("(a p) d -> p a d", p=128)
                for c in range(16):
                    nc.sync.dma_start(wvf[:, c], wv_r16[:, c])
                bv_t = vtabp.tile([128, 1024], F32)
                nc.sync.dma_start(bv_t[:], bv[:, :])
                c4 = vtabp.tile([128, 16, 256], F32)
                nc.sync.dma_start(
                    c4[:], cos4.rearrange("(a p) i -> p a i", p=128))
                s4 = vtabp.tile([128, 16, 256], F32)
                nc.sync.dma_start(
                    s4[:], sin4.rearrange("(a p) i -> p a i", p=128))

                xT_r = xT.rearrange("(a p) t -> p a t", p=128)
                for tt in range(16):
                    xa = xap.tile([128, 16, 128], F32R, tag="xa")
                    nc.sync.dma_start(
                        xa[:], xT_r[:, :, bass.ts(tt, 128)])
                    for db in range(2):
                        ds = bass.ts(db, 512)
                        ps = psp.tile([128, 512], F32, tag="ps")
                        for c in range(16):
                            nc.tensor.matmul(
                                ps[:], xa[:, c],
                                wvf[:, c, ds], start=(c == 0), stop=(c == 15))
                        vtmp = vtp.tile([128, 512], F32, tag="vtmp")
                        nc.vector.tensor_add(vtmp[:], ps[:], bv_t[:, ds])
                        v3 = vtmp[:].rearrange("p (h d) -> p h d", h=4)
                        c43 = c4[:, tt].rearrange("p (h d) -> p h d", h=4)
                        s43 = s4[:, tt].rearrange("p (h d) -> p h d", h=4)
                        me = vtp.tile([128, 4, 64], F32, tag="me")
                        mo = vtp.tile([128, 4, 64], F32, tag="mo")
                        vro = vrop.tile([128, 512], F32R, tag="vro")
                        vr3 = vro[:].rearrange("p (h d) -> p h d", h=4)
                        nc.vector.tensor_mul(
                            me[:], v3[:, :, 0:64], c43[:, :, 0:64])
                        nc.vector.tensor_mul(
                            mo[:], v3[:, :, 64:128], s43[:, :, 0:64])
                        nc.vector.tensor_sub(
                            vr3[:, :, 0:64], me[:], mo[:])
                        nc.vector.tensor_mul(
                            me[:], v3[:, :, 0:64], s43[:, :, 0:64])
                        nc.vector.tensor_mul(
                            mo[:], v3[:, :, 64:128], c43[:, :, 0:64])
                        nc.vector.tensor_add(
                            vr3[:, :, 64:128], me[:], mo[:])
                        nc.sync.dma_start(
                            vr_d[bass.ts(tt, 128), ds], vro[:])

            # ---------------- Phase B: attention per head ----------------
            with tc.tile_pool(name="hk", bufs=2) as hkp, \
                 tc.tile_pool(name="hq", bufs=2) as hqp, \
                 tc.tile_pool(name="hv", bufs=2) as hvp, \
                 tc.tile_pool(name="pt", bufs=3) as ptp, \
                 tc.tile_pool(name="bsc", bufs=3) as bscp, \
                 tc.tile_pool(name="rbb", bufs=3) as rbp, \
                 tc.tile_pool(name="oo", bufs=3) as oop, \
                 tc.tile_pool(name="const", bufs=1) as constp, \
                 tc.tile_pool(name="psB", bufs=2, space="PSUM") as psp, \
                 tc.tile_pool(name="lps", bufs=1, space="PSUM") as lpsp, \
                 tc.tile_pool(name="rps", bufs=1, space="PSUM") as rpsp, \
                 tc.tile_pool(name="ops", bufs=2, space="PSUM") as opsp:
                ones_f = constp.tile([128, 1], F32)
                nc.vector.memset(ones_f[:], 1.0)
                ones_t = constp.tile([128, 1], F32R)
                nc.vector.tensor_copy(ones_t[:], ones_f[:])
                ones_row_f = constp.tile([1, 128], F32)
                nc.vector.memset(ones_row_f[:], 1.0)
                ones_row_t = constp.tile([1, 128], F32R)
                nc.vector.tensor_copy(ones_row_t[:], ones_row_f[:])
                masks_t = constp.tile([128, 4, 512], F32)
                nc.sync.dma_start(
                    masks_t[:], masks.rearrange("p (r i) -> p r i", r=4))
                vr_r = vr_d.rearrange("(jt p) d -> p jt d", p=128)
                for h in range(HC):
                    krh = hkp.tile([128, T], F32R, tag="krh")
                    nc.sync.dma_start(
                        krh[:], qkT[1024 + h * 128:1024 + (h + 1) * 128, :])
                    qh = hqp.tile([128, T], F32R, tag="qh")
                    nc.sync.dma_start(qh[:], qkT[h * 128:(h + 1) * 128, :])
                    vh = hvp.tile([128, 16, 128], F32R, tag="vh")
                    nc.sync.dma_start(
                        vh[:], vr_r[:, :, h * 128:(h + 1) * 128])

                    for ib in range(4):
                        isl = bass.ts(ib, 512)
                        nj = 4 * ib + 4
                        l_ps = lpsp.tile([1, 512], F32, tag="l")
                        o_ps = opsp.tile([128, 512], F32, tag="o")
                        pts = [None] * nj

                        def consume(jt):
                            pt = pts[jt]
                            nc.tensor.matmul(
                                l_ps[:], ones_t[:], pt[:],
                                start=(jt == 0), stop=(jt == nj - 1))
                            nc.tensor.matmul(
                                o_ps[:], vh[:, jt], pt[:],
                                start=(jt == 0), stop=(jt == nj - 1))

                        # process key tiles in pairs: 2 score matmuls into a
                        # 2-bank PSUM tile, one batched exp over both.
                        for jp in range(nj // 2):
                            s_ps = psp.tile([128, 2, 512], F32, tag="ps")
                            for u in range(2):
                                jt = 2 * jp + u
                                nc.tensor.matmul(
                                    s_ps[:, u], krh[:, bass.ts(jt, 128)],
                                    qh[:, isl], start=True, stop=True)
                            pt2 = ptp.tile([128, 2, 512], F32R, tag="pt")
                            nc.scalar.activation(
                                pt2[:], s_ps[:],
                                mybir.ActivationFunctionType.Exp, scale=SCALE)
                            for u in range(2):
                                jt = 2 * jp + u
                                if jt >= 4 * ib:
                                    nc.vector.tensor_mul(
                                        pt2[:, u], pt2[:, u],
                                        masks_t[:, jt - 4 * ib])
                                pts[jt] = pt2[:, u]
                            if jp >= 1:
                                consume(2 * jp - 2)
                                consume(2 * jp - 1)
                        consume(nj - 2)
                        consume(nj - 1)

                        r_sb = bscp.tile([1, 512], F32, tag="r")
                        nc.vector.reciprocal_approx_fast(r_sb[:], l_ps[:])
                        rb = rbp.tile([128, 512], F32, tag="rb")
                        nc.gpsimd.partition_broadcast(rb[:], r_sb[:])
                        oo = oop.tile([128, 512], F32R, tag="oo")
                        nc.vector.tensor_mul(oo[:], o_ps[:], rb[:])
                        nc.sync.dma_start(
                            oT_d[h * 128:(h + 1) * 128, isl], oo[:])

            # ---------------- Phase C: out projection ----------------
            with tc.tile_pool(name="wpb", bufs=1) as wpp, \
                 tc.tile_pool(name="otb", bufs=2) as otbp, \
                 tc.tile_pool(name="yo", bufs=4) as yop, \
                 tc.tile_pool(name="psC", bufs=6, space="PSUM") as psp:
                wps = wpp.tile([128, 8, C], F32R)
                wp_r = wp.rearrange("(ht p) c -> p ht c", p=128)
                for ct in range(16):
                    nc.sync.dma_start(
                        wps[:, :, bass.ts(ct, 128)], wp_r[:, :, bass.ts(ct, 128)])
                oT_r = oT_d.rearrange("(ht p) t -> p ht t", p=128)
                for tb in range(4):
                    ts = bass.ts(tb, 512)
                    otb = otbp.tile([128, 8, 512], F32R, tag="otb")
                    nc.sync.dma_start(otb[:], oT_r[:, :, ts])
                    for ct in range(16):
                        ps = psp.tile([128, 512], F32, tag="ps")
                        for ht in range(8):
                            nc.tensor.matmul(
                                ps[:], wps[:, ht, bass.ts(ct, 128)],
                                otb[:, ht], start=(ht == 0), stop=(ht == 7))
                        yo = yop.tile([128, 512], F32, tag="yo")
                        nc.vector.tensor_copy(yo[:], ps[:])
                        nc.sync.dma_start(
                            yT[ct * 128:(ct + 1) * 128, ts], yo[:])

    nc.compile()
    return nc


def _prep_inputs(x, freqs_cos, freqs_sin, W_attn, b_attn, W_proj):
    """Host-side sharding / layout prep.  Returns list of 8 in_maps."""
    perm = np.concatenate([np.arange(0, HD, 2), np.arange(1, HD, 2)])

    cosT = np.ascontiguousarray(freqs_cos.T)  # [64, T]
    sinT = np.ascontiguousarray(freqs_sin.T)
    rtab_u = np.concatenate([cosT, cosT], axis=0).astype(np.float32)
    rtab_v = np.concatenate([-sinT, sinT], axis=0).astype(np.float32)
    cos4 = np.tile(freqs_cos, (1, 4)).astype(np.float32)  # [T, 256]
    sin4 = np.tile(freqs_sin, (1, 4)).astype(np.float32)

    jj = np.arange(128)[:, None]
    ii = np.arange(512)[None, :]
    masks = np.concatenate(
        [((r * 128 + jj) <= ii).astype(np.float32) for r in range(4)],
        axis=1)  # [128, 2048]

    in_maps = []
    for core in range(NCORES):
        b = core // 2
        hs = HC * (core % 2)
        cols = np.concatenate(
            [g * HD + perm for g in range(hs, hs + HC)])  # [1024]

        wqk = np.concatenate(
            [W_attn[:, cols], W_attn[:, C + cols]], axis=1)
        bqk_flat = np.concatenate([b_attn[cols], b_attn[C + cols]])
        bqk = np.ascontiguousarray(
            bqk_flat.reshape(16, 128).T)  # [128, 16], bias[jt*128+p]
        wv = W_attn[:, 2 * C + cols]
        bv = np.broadcast_to(b_attn[2 * C + cols], (128, 1024))
        wp = W_proj[cols, :]

        in_maps.append({
            "xT": np.ascontiguousarray(x[b].T).astype(np.float32),
            "wqk": np.ascontiguousarray(wqk).astype(np.float32),
            "bqk": np.ascontiguousarray(bqk).astype(np.float32),
            "wv": np.ascontiguousarray(wv).astype(np.float32),
            "bv": np.ascontiguousarray(bv).astype(np.float32),
            "wp": np.ascontiguousarray(wp).astype(np.float32),
            "rtab_u": rtab_u,
            "rtab_v": rtab_v,
            "cos4": cos4,
            "sin4": sin4,
            "masks": np.ascontiguousarray(masks),
        })
    return in_maps


def kernel(x, freqs_cos, freqs_sin, mask, W_attn, b_attn, W_proj, b_proj,
           _return_results=False, _trace=False):
    x = np.asarray(x, dtype=np.float32)
    freqs_cos = np.asarray(freqs_cos, dtype=np.float32)
    freqs_sin = np.asarray(freqs_sin, dtype=np.float32)
    W_attn = np.asarray(W_attn, dtype=np.float32)
    b_attn = np.asarray(b_attn, dtype=np.float32)
    W_proj = np.asarray(W_proj, dtype=np.float32)
    b_proj = np.asarray(b_proj, dtype=np.float32)

    if "nc" not in _CACHE:
        _CACHE["nc"] = _build_nc()
    nc = _CACHE["nc"]

    in_maps = _prep_inputs(x, freqs_cos, freqs_sin, W_attn, b_attn, W_proj)
    res = run_bass_kernel_spmd(nc, in_maps, core_ids=list(range(NCORES)),
                               trace=_trace)

    out = np.empty((B, T, C), dtype=np.float32)
    for b in range(B):
        yt0 = res.results[2 * b]["yT"]
        yt1 = res.results[2 * b + 1]["yT"]
        out[b] = yt0.T + yt1.T + b_proj[None, :]
    if _return_results:
        return out, res
    return out



# revision 12
# speedup vs baseline: 1.7037x; 1.0002x over previous
"""Causal self-attention (RoPE on k/v) TRN2 Bass kernel — v2 (bf16).

Sharding: core i handles batch b = i//2 and 8 heads hs = 8*(i%2).

v2 design vs baseline:
  - all matmul operands bf16 (same PE rate as f32r at N=512, but FWL makes
    LDWEIGHTS ~4x cheaper and halves DMA/SBUF footprints)
  - fused phase A: x^T resident once in SBUF; q/k (out [d,T]) and v (out
    [T,d]) computed from the same resident x^T; k and v SBUF-resident for
    phase B (no DRAM roundtrip); q spilled to DRAM (SBUF budget)
  - phase B: ib-outer loop, per-512-query block; scores/exp in 2-tile
    batches; softmax denom via ones-matmul; normalization chain uses
    reciprocal_approx_fast + gpsimd partition_broadcast; attention output
    written straight into SBUF-resident oT tiles
  - phase C interleaved per ib (runs on PE right behind each query block)

Head-dim permutation (even dims first) turns RoPE's interleaved pairs into
contiguous 64-wide halves; W_attn columns / W_proj rows permuted on host.
"""
import sys

sys.path.insert(0, "/opt/trn_rl_repo")

import numpy as np
import ml_dtypes

import concourse.bass as bass  # noqa: F401
import concourse.mybir as mybir
import concourse.tile as tile
from concourse import bacc
from concourse.bass_utils import run_bass_kernel_spmd

B, T, C, H = 4, 2048, 2048, 16
HD = 128
HC = 8  # heads per core
NCORES = 8
F32 = mybir.dt.float32
F32R = mybir.dt.float32r
BF16 = mybir.dt.bfloat16
SCALE = float(1.0 / np.sqrt(HD))
NPBF = ml_dtypes.bfloat16

_CACHE = {}


def _build_nc():
    nc = bacc.Bacc(num_devices=NCORES)

    xT = nc.dram_tensor("xT", [C, T], BF16, kind="ExternalInput")
    # host-swizzled so each [128,16,128] column block is one 4KB/partition
    # contiguous DMA: wqk_s[p, jt, a, c] = wqk[a*128+p, jt*128+c]
    wqk = nc.dram_tensor("wqk", [128, 16, 16, 128], BF16, kind="ExternalInput")
    bqk = nc.dram_tensor("bqk", [128, 16], F32, kind="ExternalInput")
    wv = nc.dram_tensor("wv", [C, 1024], BF16, kind="ExternalInput")
    bv = nc.dram_tensor("bv", [128, 1024], BF16, kind="ExternalInput")
    wp = nc.dram_tensor("wp", [1024, C], BF16, kind="ExternalInput")
    rtab_u = nc.dram_tensor("rtab_u", [128, T], BF16, kind="ExternalInput")
    rtab_v = nc.dram_tensor("rtab_v", [128, T], BF16, kind="ExternalInput")
    cosv = nc.dram_tensor("cosv", [128, 16, 64], BF16, kind="ExternalInput")
    sinv = nc.dram_tensor("sinv", [128, 16, 64], BF16, kind="ExternalInput")
    masks = nc.dram_tensor("masks", [128, 2048], BF16, kind="ExternalInput")
    yT = nc.dram_tensor("yT", [C, T], F32, kind="ExternalOutput")

    qT_d = nc.dram_tensor("qT_d", [1024, T], BF16)

    with tile.TileContext(nc) as tc:
        with tc.tile_pool(name="resid", bufs=1) as residp, \
             tc.tile_pool(name="bconst", bufs=1) as bcp, \
             tc.tile_pool(name="qh", bufs=3) as qhp:
            # k^T (8 head-blocks) and rope(v) stay resident through phase B
            qk_k = residp.tile([128, 8, T], BF16)
            vr = residp.tile([128, 16, 1024], BF16)
            # phase-B constants live outside the A pools so their DMAs can
            # prefetch during phase A
            masks_t = bcp.tile([128, 4, 512], BF16)
            nc.sync.dma_start(
                masks_t[:], masks.rearrange("p (r i) -> p r i", r=4))
            ones_f = bcp.tile([128, 1], F32)
            nc.vector.memset(ones_f[:], 1.0)
            ones_t = bcp.tile([128, 1], BF16)
            nc.vector.tensor_copy(ones_t[:], ones_f[:])

            # ---------------- Phase A: fused qkv projection ----------------
            with tc.tile_pool(name="xts", bufs=1) as xtp, \
                 tc.tile_pool(name="wvf", bufs=1) as wvp, \
                 tc.tile_pool(name="atab", bufs=1) as atabp, \
                 tc.tile_pool(name="wblk", bufs=2) as wbp, \
                 tc.tile_pool(name="qo", bufs=3) as qop, \
                 tc.tile_pool(name="kw", bufs=2) as kwp, \
                 tc.tile_pool(name="vw", bufs=3) as vwp:
                xT_r = xT.rearrange("(a p) t -> p a t", p=128)
                # first two weight blocks + small tables ahead of the big x
                # stream so nothing downstream waits on a queued-late DMA
                wblk0 = wbp.tile([128, 16, 128], BF16, tag="wblk")
                nc.sync.dma_start(wblk0[:], wqk[:, 0])
                wblk1 = wbp.tile([128, 16, 128], BF16, tag="wblk")
                nc.sync.dma_start(wblk1[:], wqk[:, 1])
                bqk_t = atabp.tile([128, 16], F32)
                nc.sync.dma_start(bqk_t[:], bqk[:, :])
                xts = []
                for c in range(16):
                    xc = xtp.tile([128, T], BF16, tag=f"x{c}")
                    nc.sync.dma_start(xc[:], xT_r[:, c])
                    xts.append(xc)
                # rope tables are not needed until the k/v sections
                ut = atabp.tile([128, T], BF16)
                nc.sync.dma_start(ut[:], rtab_u[:, :])
                vt_tab = atabp.tile([128, T], BF16)
                nc.sync.dma_start(vt_tab[:], rtab_v[:, :])
                c4 = atabp.tile([128, 16, 64], BF16)
                nc.sync.dma_start(c4[:], cosv[:, :, :])
                s4 = atabp.tile([128, 16, 64], BF16)
                nc.sync.dma_start(s4[:], sinv[:, :, :])
                bv_t = atabp.tile([128, 1024], BF16)
                nc.sync.dma_start(bv_t[:], bv[:, :])

                import contextlib
                _es = contextlib.ExitStack()
                psqkp = _es.enter_context(
                    tc.tile_pool(name="psqk", bufs=3, space="PSUM"))

                # --- q jt 0 prologue: chunk-major (c outer) so the PE
                # consumes x chunks as they stream in; 4 concurrent PSUM
                # groups, chunk-quads for same-bank runs.  psqk is opened
                # first so jt1+ does not wait on the prologue's PSUM frees.
                with tc.tile_pool(name="pspro", bufs=1, space="PSUM") as prop:
                    pros = {}
                    for tb in range(4):
                        pros[(0, tb)] = prop.tile(
                            [128, 512], F32, tag=f"pro0{tb}",
                            name=f"pro0{tb}")
                    for cq in range(4):
                        for tb in range(4):
                            for c in range(4 * cq, 4 * cq + 4):
                                nc.tensor.matmul(
                                    pros[(0, tb)][:], wblk0[:, c],
                                    xts[c][:, bass.ts(tb, 512)],
                                    start=(c == 0), stop=(c == 15))
                    for tb in range(4):
                        ts = bass.ts(tb, 512)
                        qo = qop.tile([128, 512], BF16, tag="qo")
                        nc.vector.tensor_scalar_add(
                            qo[:], pros[(0, tb)][:], bqk_t[:, 0:1])
                        nc.sync.dma_start(
                            qT_d[0:128, ts], qo[:])

                # --- q: out blocks jt 1..7 -> qT_d (DRAM) ---
                for jt in range(1, 8):
                    if jt == 1:
                        wblk = wblk1
                    else:
                        wblk = wbp.tile([128, 16, 128], BF16, tag="wblk")
                        nc.sync.dma_start(wblk[:], wqk[:, jt])
                    for tb in range(4):
                        ts = bass.ts(tb, 512)
                        ps = psqkp.tile([128, 512], F32, tag="ps")
                        for c in range(16):
                            nc.tensor.matmul(
                                ps[:], wblk[:, c], xts[c][:, ts],
                                start=(c == 0), stop=(c == 15))
                        qo = qop.tile([128, 512], BF16, tag="qo")
                        nc.vector.tensor_scalar_add(
                            qo[:], ps[:], bqk_t[:, jt:jt + 1])
                        nc.sync.dma_start(
                            qT_d[jt * 128:(jt + 1) * 128, ts], qo[:])

                # --- k: out blocks jt 8..15 -> rope -> qk_k (resident) ---
                for jt in range(8, 16):
                    wblk = wbp.tile([128, 16, 128], BF16, tag="wblk")
                    nc.sync.dma_start(wblk[:], wqk[:, jt])
                    for tb in range(4):
                        ts = bass.ts(tb, 512)
                        ps = psqkp.tile([128, 512], F32, tag="ps")
                        for c in range(16):
                            nc.tensor.matmul(
                                ps[:], wblk[:, c], xts[c][:, ts],
                                start=(c == 0), stop=(c == 15))
                        kt = kwp.tile([128, 512], BF16, tag="kt")
                        nc.vector.tensor_scalar_add(
                            kt[:], ps[:], bqk_t[:, jt:jt + 1])
                        kts = kwp.tile([128, 512], BF16, tag="kts")
                        nc.sync.dma_start(kts[0:64, :], kt[64:128, :])
                        nc.sync.dma_start(kts[64:128, :], kt[0:64, :])
                        nc.vector.tensor_mul(kt[:], kt[:], ut[:, ts])
                        nc.vector.tensor_mul(kts[:], kts[:], vt_tab[:, ts])
                        nc.vector.tensor_add(
                            qk_k[:, jt - 8, ts], kt[:], kts[:])

                # --- v: out blocks tt 0..15 (natural [T,d]) -> rope -> vr ---
                psvp = _es.enter_context(
                    tc.tile_pool(name="psv", bufs=4, space="PSUM"))
                wvf = wvp.tile([128, 16, 1024], BF16)
                wv_r = wv.rearrange("(a p) d -> p a d", p=128)
                for c in range(16):
                    nc.sync.dma_start(wvf[:, c], wv_r[:, c])
                for tt in range(16):
                    tsl = bass.ts(tt, 128)
                    for db in range(2):
                        ds = bass.ts(db, 512)
                        ps = psvp.tile([128, 512], F32, tag="psv")
                        for c in range(16):
                            nc.tensor.matmul(
                                ps[:], xts[c][:, tsl], wvf[:, c, ds],
                                start=(c == 0), stop=(c == 15))
                        vt_sb = vwp.tile([128, 512], BF16, tag="vt")
                        nc.vector.tensor_add(vt_sb[:], ps[:], bv_t[:, ds])
                        v3 = vt_sb[:].rearrange(
                            "p (h two d) -> p h two d", h=4, two=2)
                        cb = c4[:, tt][:, None, :].broadcast_to([128, 4, 64])
                        sb = s4[:, tt][:, None, :].broadcast_to([128, 4, 64])
                        vrv = vr[:, tt, ds].rearrange(
                            "p (h two d) -> p h two d", h=4, two=2)
                        me = vwp.tile([128, 4, 64], BF16, tag="me")
                        mo = vwp.tile([128, 4, 64], BF16, tag="mo")
                        nc.vector.tensor_mul(me[:], v3[:, :, 0], cb)
                        nc.vector.tensor_mul(mo[:], v3[:, :, 1], sb)
                        nc.vector.tensor_sub(vrv[:, :, 0], me[:], mo[:])
                        nc.vector.tensor_mul(me[:], v3[:, :, 0], sb)
                        nc.vector.tensor_mul(mo[:], v3[:, :, 1], cb)
                        nc.vector.tensor_add(vrv[:, :, 1], me[:], mo[:])
                _es.close()

            # ---------------- Phase B: software-pipelined blocks ----------------
            # Block (ib,h): emit all score MMs + exp + mask, then consume the
            # PREVIOUS block's pt tiles as two contiguous accumulation chains
            # (l then o).  Same-bank chains keep PE at the 216ns/MM rate
            # (alternating-bank accumulation measured ~310ns/MM).
            with tc.tile_pool(name="wps", bufs=1) as wpp, \
                 tc.tile_pool(name="oT", bufs=1) as oTp, \
                 tc.tile_pool(name="pt", bufs=20) as ptp, \
                 tc.tile_pool(name="rr", bufs=3) as rrp, \
                 tc.tile_pool(name="rb", bufs=3) as rbp, \
                 tc.tile_pool(name="oraw", bufs=3) as orp, \
                 tc.tile_pool(name="yo", bufs=3) as yop:
                wps = wpp.tile([128, 8, C], BF16)
                wp_r = wp.rearrange("(ht p) c -> p ht c", p=128)

                oTt = oTp.tile([128, 8, T], BF16)

                with tc.tile_pool(name="psB", bufs=3, space="PSUM") as psp, \
                     tc.tile_pool(name="lps", bufs=1, space="PSUM") as lpsp, \
                     tc.tile_pool(name="ops", bufs=1, space="PSUM") as opsp:
                    pending = None  # (pts, nj, h, isl) awaiting consume

                    def consume(pend):
                        pts, nj, h, isl = pend
                        l_ps = lpsp.tile([1, 512], F32, tag="l")
                        o_ps = opsp.tile([128, 512], F32, tag="o")
                        for jt in range(nj):
                            pt, qo = pts[jt]
                            nc.tensor.matmul(
                                l_ps[:, qo:], ones_t[:], pt,
                                start=(jt == 0), stop=(jt == nj - 1))
                        for jt in range(nj):
                            pt, qo = pts[jt]
                            nc.tensor.matmul(
                                o_ps[:, qo:], vr[:, jt, h * 128:(h + 1) * 128],
                                pt, start=(jt == 0), stop=(jt == nj - 1))
                        r_sb = rrp.tile([1, 512], F32, tag="r")
                        nc.vector.reciprocal_approx_fast(r_sb[:], l_ps[:])
                        # copy o out on the Scalar engine right away so the
                        # single o-PSUM bank frees without waiting on the
                        # reciprocal/broadcast chain
                        oraw = orp.tile([128, 512], BF16, tag="oraw")
                        nc.scalar.activation(
                            oraw[:], o_ps[:],
                            mybir.ActivationFunctionType.Copy)
                        rb = rbp.tile([128, 512], F32, tag="rb")
                        nc.gpsimd.partition_broadcast(rb[:], r_sb[:])
                        nc.vector.tensor_mul(oTt[:, h, isl], oraw[:], rb[:])

                    for ib in (3, 2, 1, 0):
                        isl = bass.ts(ib, 512)
                        nj = 4 * ib + 4
                        for h in range(HC):
                            qh = qhp.tile([128, 512], BF16, tag="qh")
                            nc.sync.dma_start(
                                qh[:], qT_d[h * 128:(h + 1) * 128, isl])
                            pts = [None] * nj
                            for jp in range(nj // 2):
                                s_ps = psp.tile([128, 2, 512], F32, tag="ps")
                                for u in range(2):
                                    jt = 2 * jp + u
                                    # causal trim: diagonal tile d only
                                    # covers queries >= 128*d in this block
                                    d = jt - 4 * ib
                                    qo = 128 * d if d > 0 else 0
                                    nc.tensor.matmul(
                                        s_ps[:, u, qo:],
                                        qk_k[:, h, bass.ts(jt, 128)],
                                        qh[:, qo:], start=True, stop=True)
                                pt2 = ptp.tile([128, 2, 512], BF16, tag="pt")
                                qop_ = max(0, 128 * (2 * jp - 4 * ib))
                                nc.scalar.activation(
                                    pt2[:, :, qop_:], s_ps[:, :, qop_:],
                                    mybir.ActivationFunctionType.Exp,
                                    scale=SCALE)
                                for u in range(2):
                                    jt = 2 * jp + u
                                    d = jt - 4 * ib
                                    qo = 128 * d if d > 0 else 0
                                    if d >= 0:
                                        # block-diagonal 128x128 triangle
                                        nc.vector.tensor_mul(
                                            pt2[:, u, qo:qo + 128],
                                            pt2[:, u, qo:qo + 128],
                                            masks_t[:, 0, 0:128])
                                    pts[jt] = (pt2[:, u, qo:], qo)
                            if pending is not None:
                                consume(pending)
                            pending = (pts, nj, h, isl)
                            if ib == 3 and h == 1:
                                # wp prefetch behind the first blocks' DMAs
                                for ht in range(8):
                                    nc.sync.dma_start(
                                        wps[:, ht], wp_r[:, ht])
                    consume(pending)

                # ---------------- Phase C: out projection ----------------
                with tc.tile_pool(name="psC", bufs=3, space="PSUM") as pcp:
                    for tb in range(4):
                        tsl = bass.ts(tb, 512)
                        for ct in range(16):
                            ps_c = pcp.tile([128, 512], F32, tag="pc")
                            for ht in range(8):
                                nc.tensor.matmul(
                                    ps_c[:], wps[:, ht, bass.ts(ct, 128)],
                                    oTt[:, ht, tsl], start=(ht == 0),
                                    stop=(ht == 7))
                            yo = yop.tile([128, 512], F32, tag="yo")
                            nc.scalar.activation(
                                yo[:], ps_c[:],
                                mybir.ActivationFunctionType.Copy)
                            nc.sync.dma_start(
                                yT[ct * 128:(ct + 1) * 128, tsl], yo[:])

    nc.compile()
    return nc


def _prep_inputs(x, freqs_cos, freqs_sin, W_attn, b_attn, W_proj):
    """Host-side sharding / layout prep.  Returns list of 8 in_maps."""
    perm = np.concatenate([np.arange(0, HD, 2), np.arange(1, HD, 2)])

    cosT = np.ascontiguousarray(freqs_cos.T)  # [64, T]
    sinT = np.ascontiguousarray(freqs_sin.T)
    rtab_u = np.concatenate([cosT, cosT], axis=0).astype(NPBF)
    rtab_v = np.concatenate([-sinT, sinT], axis=0).astype(NPBF)
    # v-rope tables in [t-partition, tt, d] layout
    cosv = np.ascontiguousarray(
        freqs_cos.reshape(16, 128, 64).transpose(1, 0, 2)).astype(NPBF)
    sinv = np.ascontiguousarray(
        freqs_sin.reshape(16, 128, 64).transpose(1, 0, 2)).astype(NPBF)

    jj = np.arange(128)[:, None]
    ii = np.arange(512)[None, :]
    masks = np.concatenate(
        [((r * 128 + jj) <= ii).astype(np.float32) for r in range(4)],
        axis=1).astype(NPBF)  # [128, 2048]

    in_maps = []
    for core in range(NCORES):
        b = core // 2
        hs = HC * (core % 2)
        cols = np.concatenate(
            [g * HD + perm for g in range(hs, hs + HC)])  # [1024]

        wqk_h = np.concatenate(
            [W_attn[:, cols], W_attn[:, C + cols]], axis=1)
        # swizzle: [C, 2048] -> [128p, 16jt, 16a, 128c]
        wqk_h = wqk_h.reshape(16, 128, 16, 128).transpose(1, 2, 0, 3)
        bqk_flat = np.concatenate([b_attn[cols], b_attn[C + cols]])
        bqk_h = np.ascontiguousarray(
            bqk_flat.reshape(16, 128).T)  # [128, 16], bias[jt*128+p]
        wv_h = W_attn[:, 2 * C + cols]
        bv_h = np.broadcast_to(b_attn[2 * C + cols], (128, 1024))
        wp_h = W_proj[cols, :]

        in_maps.append({
            "xT": np.ascontiguousarray(x[b].T).astype(NPBF),
            "wqk": np.ascontiguousarray(wqk_h).astype(NPBF),
            "bqk": np.ascontiguousarray(bqk_h).astype(np.float32),
            "wv": np.ascontiguousarray(wv_h).astype(NPBF),
            "bv": np.ascontiguousarray(bv_h).astype(NPBF),
            "wp": np.ascontiguousarray(wp_h).astype(NPBF),
            "rtab_u": rtab_u,
            "rtab_v": rtab_v,
            "cosv": cosv,
            "sinv": sinv,
            "masks": np.ascontiguousarray(masks),
        })
    return in_maps


def kernel(x, freqs_cos, freqs_sin, mask, W_attn, b_attn, W_proj, b_proj,
           _return_results=False, _trace=False):
    x = np.asarray(x, dtype=np.float32)
    freqs_cos = np.asarray(freqs_cos, dtype=np.float32)
    freqs_sin = np.asarray(freqs_sin, dtype=np.float32)
    W_attn = np.asarray(W_attn, dtype=np.float32)
    b_attn = np.asarray(b_attn, dtype=np.float32)
    W_proj = np.asarray(W_proj, dtype=np.float32)
    b_proj = np.asarray(b_proj, dtype=np.float32)

    if "nc" not in _CACHE:
        _CACHE["nc"] = _build_nc()
    nc = _CACHE["nc"]

    in_maps = _prep_inputs(x, freqs_cos, freqs_sin, W_attn, b_attn, W_proj)
    res = run_bass_kernel_spmd(nc, in_maps, core_ids=list(range(NCORES)),
                               trace=_trace)

    out = np.empty((B, T, C), dtype=np.float32)
    for b in range(B):
        yt0 = res.results[2 * b]["yT"]
        yt1 = res.results[2 * b + 1]["yT"]
        out[b] = yt0.T + yt1.T + b_proj[None, :]
    if _return_results:
        return out, res
    return out
